# revision 5
# baseline (speedup 1.0000x reference)
"""Self-contained TRN2 Bass kernel for nn_EuclideanSimilarity.

Full-input contract: kernel(x, W, b) with
  x [4, 4096, 128] f32, W [128, 128] f32, b [128] f32
returns out [4, 4096, 4096] f32 = exp(-pairwise_euclidean_dist(x @ W.T + b)).

Sharding: 8 cores, core c -> (batch c//2, query-half c%2); each core computes
its [2048, 4096] block of the pairwise-similarity matrix against the full
key set of its batch (SPMD — identical program, different input slices).

Per-core pipeline: transpose x via PE, hT = W @ xT + b (fp32r matmul),
then d2[m,n] = sq[m] + sq[n] - 2 h_m.h_n assembled entirely in PSUM.
The PE's fast fp32r path only carries ~12 mantissa bits, which would
destroy the near-diagonal cancellation, so both the gram term and the
squared-norm rows use hi/lo split-precision (x = hi + lo, each fp32r):
gram = g_hi.k_hi + g_hi.k_lo + g_lo.k_hi (with g = -2h), and the norm
rows enter via an augmented K=4 matmul with hi/lo rows. PSUM is then
drained by DVE with a fused relu, sqrt and exp(-x) run on the scalar
engine (grouped by activation-table set to avoid table thrash), and each
128-row output tile leaves via one 2 MiB contiguous DMA.
"""

from contextlib import ExitStack

import numpy as np

import concourse.mybir as mybir
import concourse.tile as tile
from concourse import bacc
from concourse.bass import ts
from concourse.masks import make_identity

F32 = mybir.dt.float32
F32R = mybir.dt.float32r
AF = mybir.ActivationFunctionType
ALU = mybir.AluOpType

B = 4
N = 4096
NQ = 2048  # query rows per core
NK = 4096  # key rows per core
D = 128
TEMPERATURE = 1.0
G = 5  # qtiles per sqrt/exp batch (ACT table-set grouping)
NQT = NQ // 128  # query tiles per core
N_CORES = 8


def kernel_body(ctx: ExitStack, tc: tile.TileContext, out, xq, xk, W, b):
    nc = tc.nc

    consts = ctx.enter_context(tc.tile_pool(name="consts", bufs=1))
    ident = consts.tile([128, 128], F32)
    make_identity(nc, ident[:])

    w_sb = consts.tile([128, 128], F32)
    nc.sync.dma_start(w_sb[:], W[:, :])
    b_sb = consts.tile([128, 1], F32)
    nc.sync.dma_start(b_sb[:], b[:, :])
    bm2_sb = consts.tile([128, 1], F32)
    nc.scalar.mul(bm2_sb[:], b_sb[:], -2.0)
    ones_f32 = consts.tile([128, 512], F32)
    nc.gpsimd.memset(ones_f32[:], 1.0)
    ones_col = consts.tile([128, 1], F32R)   # lhsT for sq matmuls
    nc.vector.tensor_copy(ones_col[:], ones_f32[:, 0:1])
    ones_row = consts.tile([1, 512], F32R)   # ones chunks for aug rows
    nc.vector.tensor_copy(ones_row[:], ones_f32[0:1, :])

    # persistent main-loop operands (hi/lo split for fp32-grade gram)
    h_pool = ctx.enter_context(tc.tile_pool(name="h", bufs=1))
    hk_hi = h_pool.tile([128, NK], F32R)
    hk_lo = h_pool.tile([128, NK], F32R)
    gq_hi = h_pool.tile([128, NQ], F32R)   # g = -2*h (queries)
    gq_lo = h_pool.tile([128, NQ], F32R)

    aug_pool = ctx.enter_context(tc.tile_pool(name="aug", bufs=1))
    # pairing: d2 += sum_k aug_q[k,m] * aug_k[k,n]
    #   k=0: 1 * sq_k_hi   k=1: 1 * sq_k_lo
    #   k=2: sq_q_hi * 1   k=3: sq_q_lo * 1
    aug_k = aug_pool.tile([4, NK], F32R)   # rows: sq_k_hi, sq_k_lo, 1, 1
    aug_q = aug_pool.tile([4, NQ], F32R)   # rows: 1, 1, sq_q_hi, sq_q_lo

    xk_r = xk.rearrange("(t p) d -> p t d", p=128)
    xq_r = xq.rearrange("(t p) d -> p t d", p=128)

    # ---------------- setup phase (scoped pools) ----------------
    with tc.tile_pool(name="setup_sb", bufs=3) as ssb, \
         tc.tile_pool(name="setup_ps", bufs=3, space="PSUM") as sps, \
         tc.tile_pool(name="rows", bufs=1) as rows_pool:

        wt_ps = sps.tile([128, 512], F32, tag="ps")
        nc.tensor.transpose(wt_ps[:, 0:128], w_sb[:], ident[:])
        wt_sb = consts.tile([128, 128], F32R)
        nc.vector.tensor_copy(wt_sb[:], wt_ps[:, 0:128])

        # single-partition staging rows, later DMAed onto aug partitions 1-3
        sqk_lo_row = rows_pool.tile([1, NK], F32R)
        sqq_hi_row = rows_pool.tile([1, NQ], F32R)
        sqq_lo_row = rows_pool.tile([1, NQ], F32R)
        ones_nk = rows_pool.tile([1, NK], F32R)
        for c in range(NK // 512):
            nc.vector.tensor_copy(ones_nk[0:1, ts(c, 512)], ones_row[:])

        def do_chunks(nchunks, x_r, hi_dst, lo_dst, is_q):
            for c in range(nchunks):
                tagn = "q" if is_q else "k"
                xin = ssb.tile([128, 512], F32, tag="xin", name=f"xin_{tagn}{c}")
                nc.sync.dma_start(
                    xin[:].rearrange("p (t d) -> p t d", d=D),
                    x_r[:, 4 * c:4 * c + 4, :],
                )
                tp = sps.tile([128, 512], F32, tag="ps", name=f"tp_{tagn}{c}")
                for j in range(4):
                    nc.tensor.transpose(
                        tp[:, ts(j, 128)], xin[:, ts(j, 128)], ident[:]
                    )
                xt = ssb.tile([128, 512], F32R, tag="xt", name=f"xt_{tagn}{c}")
                nc.vector.tensor_copy(xt[:], tp[:])
                hps = sps.tile([128, 512], F32, tag="ps", name=f"hps_{tagn}{c}")
                nc.tensor.matmul(hps[:], wt_sb[:], xt[:], start=True, stop=True)
                hf = ssb.tile([128, 512], F32, tag="hf", name=f"hf_{tagn}{c}")
                if is_q:  # g = -2*(W@xT) - 2b
                    nc.scalar.activation(
                        hf[:], hps[:], AF.Identity, bias=bm2_sb[:, 0:1],
                        scale=-2.0,
                    )
                else:
                    nc.scalar.activation(
                        hf[:], hps[:], AF.Identity, bias=b_sb[:, 0:1]
                    )
                # hi/lo split of h (or g)
                nc.vector.tensor_copy(hi_dst[:, ts(c, 512)], hf[:])
                nc.vector.tensor_tensor(
                    lo_dst[:, ts(c, 512)], hf[:], hi_dst[:, ts(c, 512)],
                    ALU.subtract,
                )
                # squared norms, also hi/lo so the K=128 sum keeps f32 grade
                s2f = ssb.tile([128, 512], F32, tag="s2f", name=f"s2f_{tagn}{c}")
                nc.vector.tensor_mul(s2f[:], hf[:], hf[:])
                s2hi = ssb.tile([128, 512], F32R, tag="s2hi", name=f"s2hi_{tagn}{c}")
                nc.vector.tensor_copy(s2hi[:], s2f[:])
                s2lo = ssb.tile([128, 512], F32R, tag="s2lo", name=f"s2lo_{tagn}{c}")
                nc.vector.tensor_tensor(s2lo[:], s2f[:], s2hi[:], ALU.subtract)
                sqps = sps.tile([128, 512], F32, tag="ps", name=f"sqps_{tagn}{c}")
                nc.tensor.matmul(
                    sqps[0:1, :], ones_col[:], s2hi[:], start=True, stop=False
                )
                nc.tensor.matmul(
                    sqps[0:1, :], ones_col[:], s2lo[:], start=False, stop=True
                )
                if is_q:  # s2 = 4*h^2 -> scale by 1/4
                    nc.vector.tensor_scalar_mul(
                        sqq_hi_row[0:1, ts(c, 512)], sqps[0:1, :], 0.25
                    )
                    nc.vector.scalar_tensor_tensor(
                        sqq_lo_row[0:1, ts(c, 512)], sqps[0:1, :], 0.25,
                        sqq_hi_row[0:1, ts(c, 512)], ALU.mult, ALU.subtract,
                    )
                else:
                    nc.vector.tensor_copy(aug_k[0:1, ts(c, 512)], sqps[0:1, :])
                    nc.vector.tensor_tensor(
                        sqk_lo_row[0:1, ts(c, 512)], sqps[0:1, :],
                        aug_k[0:1, ts(c, 512)], ALU.subtract,
                    )

        do_chunks(NK // 512, xk_r, hk_hi, hk_lo, False)
        do_chunks(NQ // 512, xq_r, gq_hi, gq_lo, True)

        # assemble remaining aug partitions via SBUF->SBUF DMA
        nc.sync.dma_start(aug_k[1:2, :], sqk_lo_row[:])
        nc.sync.dma_start(aug_k[2:3, :], ones_nk[:])
        nc.sync.dma_start(aug_k[3:4, :], ones_nk[:])
        for c in range(NQ // 512):
            nc.vector.tensor_copy(aug_q[0:1, ts(c, 512)], ones_row[:])
        nc.sync.dma_start(aug_q[1:2, :], ones_nk[0:1, 0:NQ])
        nc.sync.dma_start(aug_q[2:3, :], sqq_hi_row[:])
        nc.sync.dma_start(aug_q[3:4, :], sqq_lo_row[:])

    # ---------------- main loop ----------------
    stage_pool = ctx.enter_context(tc.tile_pool(name="stage", bufs=G + 1))
    d2_ps = ctx.enter_context(tc.tile_pool(name="d2", bufs=8, space="PSUM"))
    NC = NK // 512  # 8 key chunks

    for g0 in range(0, NQT, G):
        group = []
        for qt in range(g0, min(g0 + G, NQT)):
            pss = [
                d2_ps.tile([128, 512], F32, tag="d2", name=f"d2_{qt}_{c}")
                for c in range(NC)
            ]
            # weight-group order: gq_hi serves both hk_hi and hk_lo passes
            for c in range(NC):
                nc.tensor.matmul(
                    pss[c][:], gq_hi[:, ts(qt, 128)], hk_hi[:, ts(c, 512)],
                    start=True, stop=False,
                )
            for c in range(NC):
                nc.tensor.matmul(
                    pss[c][:], gq_hi[:, ts(qt, 128)], hk_lo[:, ts(c, 512)],
                    start=False, stop=False,
                )
            for c in range(NC):
                nc.tensor.matmul(
                    pss[c][:], gq_lo[:, ts(qt, 128)], hk_hi[:, ts(c, 512)],
                    start=False, stop=False,
                )
            for c in range(NC):
                nc.tensor.matmul(
                    pss[c][:], aug_q[:, ts(qt, 128)], aug_k[:, ts(c, 512)],
                    start=False, stop=True,
                )
            st = stage_pool.tile([128, NK], F32, tag="st", name=f"st{qt}")
            for c in range(NC):
                nc.vector.tensor_scalar_max(st[:, ts(c, 512)], pss[c][:], 0.0)
            nc.scalar.activation(st[:], st[:], AF.Sqrt)
            group.append((qt, st))
        for qt, st in group:
            nc.scalar.activation(st[:], st[:], AF.Exp, scale=-TEMPERATURE)
            nc.sync.dma_start(out[ts(qt, 128), :], st[:])


def build_nc():
    nc = bacc.Bacc("TRN2", target_bir_lowering=False, debug=False)
    xq = nc.dram_tensor("xq", [NQ, D], F32, kind="ExternalInput").ap()
    xk = nc.dram_tensor("xk", [NK, D], F32, kind="ExternalInput").ap()
    W = nc.dram_tensor("W", [D, D], F32, kind="ExternalInput").ap()
    b = nc.dram_tensor("b", [D, 1], F32, kind="ExternalInput").ap()
    out = nc.dram_tensor("out", [NQ, NK], F32, kind="ExternalOutput").ap()
    with tile.TileContext(nc) as tc:
        with ExitStack() as ctx:
            kernel_body(ctx, tc, out, xq, xk, W, b)
    nc.compile()
    return nc


_NC_CACHE = None


def _get_nc():
    global _NC_CACHE
    if _NC_CACHE is None:
        _NC_CACHE = build_nc()
    return _NC_CACHE


def _run(x, W, b, trace=False, **spmd_kwargs):
    from concourse.bass_utils import run_bass_kernel_spmd

    x = np.asarray(x, dtype=np.float32)
    W = np.asarray(W, dtype=np.float32)
    b = np.asarray(b, dtype=np.float32).reshape(D, 1)
    nc = _get_nc()
    in_maps = []
    for c in range(N_CORES):
        bi, qh = c // 2, c % 2
        in_maps.append({
            "xq": np.ascontiguousarray(x[bi, qh * NQ:(qh + 1) * NQ, :]),
            "xk": np.ascontiguousarray(x[bi]),
            "W": W,
            "b": b,
        })
    res = run_bass_kernel_spmd(
        nc, in_maps, core_ids=list(range(N_CORES)), trace=trace, **spmd_kwargs
    )
    out = np.empty((B, N, N), dtype=np.float32)
    for c in range(N_CORES):
        bi, qh = c // 2, c % 2
        out[bi, qh * NQ:(qh + 1) * NQ, :] = res.results[c]["out"]
    return out, res


def kernel(x, W, b):
    out, _ = _run(x, W, b)
    return out


# revision 35
# speedup vs baseline: 377.3951x; 377.3951x over previous
"""Self-contained TRN2 Bass kernel for nn_EuclideanSimilarity.

Full-input contract: kernel(x, W, b) with
  x [4, 4096, 128] f32, W [128, 128] f32, b [128] f32
returns out [4, 4096, 4096] f32 = exp(-pairwise_euclidean_dist(x @ W.T + b)).

Sharding: 8 cores, core c -> (batch c//2, query-half c%2); each core computes
its [2048, 4096] block of the pairwise-similarity matrix against the full
key set of its batch (SPMD — identical program, different input slices).

Per-core pipeline: transpose x via PE, hT = W @ xT + b (fp32r matmul),
then d2[m,n] = sq[m] + sq[n] - 2 h_m.h_n assembled in PSUM. The PE's
fast fp32r path only carries ~12 mantissa bits, which would destroy the
near-diagonal cancellation, so the gram term uses hi/lo split-precision
(h = hi + lo, each fp32r): gram = g_hi.k_hi + g_hi.k_lo + g_lo.k_hi
(with g = -2h), the key norms enter via an augmented K=2 matmul with
fp32r hi/lo rows, and the query norm is added at full fp32 by the DVE
drain (tensor_scalar: out = max(psum + sq_q[m], 0), which also fuses the
relu while evacuating PSUM). sqrt and exp(-x) run on the scalar engine,
explicitly order-chained in batches so the sqrt/exp activation-table
sets are not thrashed, and each 128-row output tile leaves through one
2 MiB contiguous DMA. PSUM cycles through 4x[128,1024] slots, each
completed by 8 back-to-back matmuls, to keep the PE clock-gate warm.
"""

from contextlib import ExitStack

import numpy as np

import concourse.mybir as mybir
import concourse.tile as tile
from concourse.tile import add_dep_helper
from concourse import bacc
from concourse.bass import ts
from concourse.masks import make_identity

F32 = mybir.dt.float32
F32R = mybir.dt.float32r
AF = mybir.ActivationFunctionType
ALU = mybir.AluOpType

B = 4
N = 4096
NQ = 2048  # query rows per core
NK = 4096  # key rows per core
D = 128
TEMPERATURE = 1.0
NQT = NQ // 128  # query tiles per core
N_CORES = 8


def kernel_body(ctx: ExitStack, tc: tile.TileContext, out, xq, xk, W, b):
    nc = tc.nc

    consts = ctx.enter_context(tc.tile_pool(name="consts", bufs=1))
    ident = consts.tile([128, 128], F32)
    make_identity(nc, ident[:])

    w_sb = consts.tile([128, 128], F32)
    nc.sync.dma_start(w_sb[:], W[:, :])
    b_sb = consts.tile([128, 1], F32)
    nc.sync.dma_start(b_sb[:], b[:, :])
    bm2_sb = consts.tile([128, 1], F32)
    nc.scalar.mul(bm2_sb[:], b_sb[:], -2.0)
    ones_f32 = consts.tile([128, 512], F32)
    nc.gpsimd.memset(ones_f32[:], 1.0)
    ones_col = consts.tile([128, 1], F32)    # lhsT for the f32 sq matmul
    nc.vector.tensor_copy(ones_col[:], ones_f32[:, 0:1])

    # persistent main-loop operands (hi/lo split for fp32-grade gram)
    h_pool = ctx.enter_context(tc.tile_pool(name="h", bufs=1))
    hk_hi = h_pool.tile([128, NK], F32R)
    hk_lo = h_pool.tile([128, NK], F32R)
    gq_hi = h_pool.tile([128, NQ], F32R)   # g = -2*h (queries)
    gq_lo = h_pool.tile([128, NQ], F32R)

    aug_pool = ctx.enter_context(tc.tile_pool(name="aug", bufs=1))
    # d2 += sum_k ones2[k,m] * aug_k[k,n] = sq_k_hi[n] + sq_k_lo[n];
    # sq_q[m] is added per-partition by the DVE relu (full fp32, no split)
    aug_k = aug_pool.tile([2, NK], F32R)   # rows: sq_k_hi, sq_k_lo
    ones2 = aug_pool.tile([2, 128], F32R)  # constant lhsT for the aug matmul
    nc.vector.tensor_copy(ones2[:], ones_f32[0:2, 0:128])
    sqq_cols = aug_pool.tile([128, NQT], F32)  # sq_q in column-per-qtile form

    xk_r = xk.rearrange("(t p) d -> p t d", p=128)
    xq_r = xq.rearrange("(t p) d -> p t d", p=128)

    # ---------------- setup phase (scoped pools) ----------------
    with tc.tile_pool(name="setup_sb", bufs=6) as ssb, \
         tc.tile_pool(name="setup_ps", bufs=2, space="PSUM") as sps, \
         tc.tile_pool(name="rows", bufs=1) as rows_pool:

        wt_ps = sps.tile([128, 512], F32, tag="wt", bufs=1)
        nc.tensor.transpose(wt_ps[:, 0:128], w_sb[:], ident[:])
        wt_sb = consts.tile([128, 128], F32R)
        nc.vector.tensor_copy(wt_sb[:], wt_ps[:, 0:128])

        # single-partition staging row for raw query norms (fp32, 4*|h|^2)
        sqq_row = rows_pool.tile([1, NQ], F32)

        def do_chunks(nchunks, x_r, hi_dst, lo_dst, is_q):
            for c in range(nchunks):
                tagn = "q" if is_q else "k"
                xin = ssb.tile([128, 512], F32, tag="xin", name=f"xin_{tagn}{c}")
                nc.sync.dma_start(
                    xin[:].rearrange("p (t d) -> p t d", d=D),
                    x_r[:, 4 * c:4 * c + 4, :],
                )
                tp = sps.tile([128, 512], F32, tag="tp", bufs=3, name=f"tp_{tagn}{c}")
                for j in range(4):
                    nc.tensor.transpose(
                        tp[:, ts(j, 128)], xin[:, ts(j, 128)], ident[:]
                    )
                xt = ssb.tile([128, 512], F32R, tag="xt", name=f"xt_{tagn}{c}")
                nc.scalar.activation(xt[:], tp[:], AF.Identity)
                hps = sps.tile([128, 512], F32, tag="hps", bufs=2, name=f"hps_{tagn}{c}")
                nc.tensor.matmul(hps[:], wt_sb[:], xt[:], start=True, stop=True)
                hf = ssb.tile([128, 512], F32, tag="hf", name=f"hf_{tagn}{c}")
                if is_q:  # g = -2*(W@xT) - 2b
                    nc.scalar.activation(
                        hf[:], hps[:], AF.Identity, bias=bm2_sb[:, 0:1],
                        scale=-2.0,
                    )
                else:
                    nc.scalar.activation(
                        hf[:], hps[:], AF.Identity, bias=b_sb[:, 0:1]
                    )
                # hi/lo split of h (or g)
                nc.gpsimd.tensor_copy(hi_dst[:, ts(c, 512)], hf[:])
                nc.vector.tensor_tensor(
                    lo_dst[:, ts(c, 512)], hf[:], hi_dst[:, ts(c, 512)],
                    ALU.subtract,
                )
                # squared norms, also hi/lo so the K=128 sum keeps f32 grade
                s2f = ssb.tile([128, 512], F32, tag="s2f", name=f"s2f_{tagn}{c}")
                nc.vector.tensor_mul(s2f[:], hf[:], hf[:])
                sqps = sps.tile([128, 512], F32, tag="sqps", bufs=2, name=f"sqps_{tagn}{c}")
                # plain-f32 matmul (2-pass internally) keeps the norm exact
                nc.tensor.matmul(
                    sqps[0:1, :], ones_col[:], s2f[:], start=True, stop=True
                )
                if is_q:  # raw 4*|h|^2; the 1/4 scale is applied at transpose
                    nc.scalar.activation(
                        sqq_row[0:1, ts(c, 512)], sqps[0:1, :], AF.Identity
                    )
                else:
                    nc.scalar.activation(
                        aug_k[0:1, ts(c, 512)], sqps[0:1, :], AF.Identity
                    )
                    sk = rows_pool.tile([1, 512], F32R, tag="sklo", bufs=2,
                                        name=f"sklo{c}")
                    nc.vector.tensor_tensor(
                        sk[:], sqps[0:1, :], aug_k[0:1, ts(c, 512)],
                        ALU.subtract,
                    )
                    nc.sync.dma_start(aug_k[1:2, ts(c, 512)], sk[:])

        do_chunks(NQ // 512, xq_r, gq_hi, gq_lo, True)
        # transpose sq_q row into column-per-qtile layout via tiny PE transposes
        sqq_ps = sps.tile([128, 512], F32, tag="sqps", bufs=2, name="sqq_ps")
        for qt in range(NQT):
            nc.tensor.transpose(
                sqq_ps[:, qt:qt + 1], sqq_row[0:1, ts(qt, 128)], ident[0:1, 0:1]
            )
        nc.vector.tensor_scalar_mul(sqq_cols[:], sqq_ps[:, 0:NQT], 0.25)
        do_chunks(NK // 512, xk_r, hk_hi, hk_lo, False)

    # ---------------- main loop ----------------
    stage_pool = ctx.enter_context(tc.tile_pool(name="stage", bufs=8))
    d2_ps = ctx.enter_context(tc.tile_pool(name="d2", bufs=4, space="PSUM"))
    NC = NK // 512  # 8 key chunks

    last_act = [None]

    def chained_act(*args, chain=True, **kwargs):
        bi = nc.scalar.activation(*args, **kwargs)
        if chain and last_act[0] is not None:
            # arg order: (waiter, dependency) - this op waits on the previous
            add_dep_helper(bi.ins, last_act[0].ins, sync=False,
                           reason="act-table-order")
        last_act[0] = bi
        return bi

    NH = NK // 1024  # 4 slots of 2 key-chunks each
    spans = [(0, 3), (3, 6), (6, 9), (9, 12), (12, 14), (14, 16)]
    assert spans[-1][1] == NQT
    for g0, g1 in spans:
        group = []
        for qt in range(g0, g1):
            st = stage_pool.tile([128, NK], F32, tag="st", name=f"st{qt}")
            for h in range(NH):
                ps = d2_ps.tile([128, 1024], F32, tag="d2", name=f"d2_{qt}_{h}")
                # chunk-major: finish each 512-column before moving on, so the
                # slot completes after 8 back-to-back matmuls
                for cc in range(2):
                    c = 2 * h + cc
                    nc.tensor.matmul(
                        ps[:, ts(cc, 512)], gq_hi[:, ts(qt, 128)],
                        hk_hi[:, ts(c, 512)], start=True, stop=False,
                    )
                    nc.tensor.matmul(
                        ps[:, ts(cc, 512)], gq_hi[:, ts(qt, 128)],
                        hk_lo[:, ts(c, 512)], start=False, stop=False,
                    )
                    nc.tensor.matmul(
                        ps[:, ts(cc, 512)], gq_lo[:, ts(qt, 128)],
                        hk_hi[:, ts(c, 512)], start=False, stop=False,
                    )
                    nc.tensor.matmul(
                        ps[:, ts(cc, 512)], ones2[:],
                        aug_k[:, ts(c, 512)], start=False, stop=True,
                    )
                nc.vector.tensor_scalar(
                    st[:, ts(h, 1024)], ps[:], sqq_cols[:, qt:qt + 1], 0.0,
                    ALU.add, ALU.max,
                )
            if g0 == 0 and qt < 3:
                # pipeline-fill phase: sqrt per 2048-half starts ~2 slots earlier
                chained_act(st[:, 0:2048], st[:, 0:2048], AF.Sqrt)
                chained_act(st[:, 2048:4096], st[:, 2048:4096], AF.Sqrt)
            else:
                chained_act(st[:], st[:], AF.Sqrt)
            group.append((qt, st))
        for qt, st in group:
            chained_act(st[:], st[:], AF.Exp, scale=-TEMPERATURE)
            nc.sync.dma_start(out[ts(qt, 128), :], st[:])


def build_nc():
    nc = bacc.Bacc("TRN2", target_bir_lowering=False, debug=False)
    xq = nc.dram_tensor("xq", [NQ, D], F32, kind="ExternalInput").ap()
    xk = nc.dram_tensor("xk", [NK, D], F32, kind="ExternalInput").ap()
    W = nc.dram_tensor("W", [D, D], F32, kind="ExternalInput").ap()
    b = nc.dram_tensor("b", [D, 1], F32, kind="ExternalInput").ap()
    out = nc.dram_tensor("out", [NQ, NK], F32, kind="ExternalOutput").ap()
    with tile.TileContext(nc) as tc:
        with ExitStack() as ctx:
            kernel_body(ctx, tc, out, xq, xk, W, b)
    nc.compile()
    return nc


_NC_CACHE = None


def _get_nc():
    global _NC_CACHE
    if _NC_CACHE is None:
        _NC_CACHE = build_nc()
    return _NC_CACHE


def _run(x, W, b, trace=False, **spmd_kwargs):
    from concourse.bass_utils import run_bass_kernel_spmd

    x = np.asarray(x, dtype=np.float32)
    W = np.asarray(W, dtype=np.float32)
    b = np.asarray(b, dtype=np.float32).reshape(D, 1)
    nc = _get_nc()
    in_maps = []
    for c in range(N_CORES):
        bi, qh = c // 2, c % 2
        in_maps.append({
            "xq": np.ascontiguousarray(x[bi, qh * NQ:(qh + 1) * NQ, :]),
            "xk": np.ascontiguousarray(x[bi]),
            "W": W,
            "b": b,
        })
    res = run_bass_kernel_spmd(
        nc, in_maps, core_ids=list(range(N_CORES)), trace=trace, **spmd_kwargs
    )
    out = np.empty((B, N, N), dtype=np.float32)
    for c in range(N_CORES):
        bi, qh = c // 2, c % 2
        out[bi, qh * NQ:(qh + 1) * NQ, :] = res.results[c]["out"]
    return out, res


def kernel(x, W, b):
    out, _ = _run(x, W, b)
    return out


# revision 38
# speedup vs baseline: 377.4956x; 1.0003x over previous
"""Self-contained TRN2 Bass kernel for nn_EuclideanSimilarity.

Full-input contract: kernel(x, W, b) with
  x [4, 4096, 128] f32, W [128, 128] f32, b [128] f32
returns out [4, 4096, 4096] f32 = exp(-pairwise_euclidean_dist(x @ W.T + b)).

Sharding: 8 cores, core c -> (batch c//2, query-half c%2); each core computes
its [2048, 4096] block of the pairwise-similarity matrix against the full
key set of its batch (SPMD — identical program, different input slices).

Per-core pipeline: transpose x via PE, hT = W @ xT + b (fp32r matmul),
then d2[m,n] = sq[m] + sq[n] - 2 h_m.h_n assembled in PSUM. The PE's
fast fp32r path only carries ~12 mantissa bits, which would destroy the
near-diagonal cancellation, so the gram term uses hi/lo split-precision
(h = hi + lo, each fp32r): gram = g_hi.k_hi + g_hi.k_lo + g_lo.k_hi
(with g = -2h), the key norms enter via an augmented K=2 matmul with
fp32r hi/lo rows, and the query norm is added at full fp32 by the DVE
drain (tensor_scalar: out = max(psum + sq_q[m], 0), which also fuses the
relu while evacuating PSUM). sqrt and exp(-x) run on the scalar engine,
explicitly order-chained in batches so the sqrt/exp activation-table
sets are not thrashed, and each 128-row output tile leaves through one
2 MiB contiguous DMA. PSUM cycles through 4x[128,1024] slots, each
completed by 8 back-to-back matmuls, to keep the PE clock-gate warm.
"""

from contextlib import ExitStack

import numpy as np

import concourse.mybir as mybir
import concourse.tile as tile
from concourse.tile import add_dep_helper
from concourse import bacc
from concourse.bass import ts
from concourse.masks import make_identity

F32 = mybir.dt.float32
F32R = mybir.dt.float32r
AF = mybir.ActivationFunctionType
ALU = mybir.AluOpType

B = 4
N = 4096
NQ = 2048  # query rows per core
NK = 4096  # key rows per core
D = 128
TEMPERATURE = 1.0
NQT = NQ // 128  # query tiles per core
N_CORES = 8


def kernel_body(ctx: ExitStack, tc: tile.TileContext, out, xq, xk, W, b):
    nc = tc.nc

    consts = ctx.enter_context(tc.tile_pool(name="consts", bufs=1))
    ident = consts.tile([128, 128], F32)
    make_identity(nc, ident[:])

    w_sb = consts.tile([128, 128], F32)
    nc.sync.dma_start(w_sb[:], W[:, :])
    b_sb = consts.tile([128, 1], F32)
    nc.sync.dma_start(b_sb[:], b[:, :])
    bm2_sb = consts.tile([128, 1], F32)
    nc.scalar.mul(bm2_sb[:], b_sb[:], -2.0)
    ones_f32 = consts.tile([128, 512], F32)
    nc.gpsimd.memset(ones_f32[:], 1.0)
    ones_col = consts.tile([128, 1], F32)    # lhsT for the f32 sq matmul
    nc.vector.tensor_copy(ones_col[:], ones_f32[:, 0:1])

    # persistent main-loop operands (hi/lo split for fp32-grade gram)
    h_pool = ctx.enter_context(tc.tile_pool(name="h", bufs=1))
    hk_hi = h_pool.tile([128, NK], F32R)
    hk_lo = h_pool.tile([128, NK], F32R)
    gq_hi = h_pool.tile([128, NQ], F32R)   # g = -2*h (queries)
    gq_lo = h_pool.tile([128, NQ], F32R)

    aug_pool = ctx.enter_context(tc.tile_pool(name="aug", bufs=1))
    # d2 += sum_k ones2[k,m] * aug_k[k,n] = sq_k_hi[n] + sq_k_lo[n];
    # sq_q[m] is added per-partition by the DVE relu (full fp32, no split)
    aug_k = aug_pool.tile([2, NK], F32R)   # rows: sq_k_hi, sq_k_lo
    ones2 = aug_pool.tile([2, 128], F32R)  # constant lhsT for the aug matmul
    nc.vector.tensor_copy(ones2[:], ones_f32[0:2, 0:128])
    sqq_cols = aug_pool.tile([128, NQT], F32)  # sq_q in column-per-qtile form

    xk_r = xk.rearrange("(t p) d -> p t d", p=128)
    xq_r = xq.rearrange("(t p) d -> p t d", p=128)

    # ---------------- setup phase (scoped pools) ----------------
    with tc.tile_pool(name="setup_sb", bufs=6) as ssb, \
         tc.tile_pool(name="setup_ps", bufs=2, space="PSUM") as sps, \
         tc.tile_pool(name="rows", bufs=1) as rows_pool:

        wt_ps = sps.tile([128, 512], F32, tag="wt", bufs=1)
        nc.tensor.transpose(wt_ps[:, 0:128], w_sb[:], ident[:])
        wt_sb = consts.tile([128, 128], F32R)
        nc.vector.tensor_copy(wt_sb[:], wt_ps[:, 0:128])

        # single-partition staging row for raw query norms (fp32, 4*|h|^2)
        sqq_row = rows_pool.tile([1, NQ], F32)

        def do_chunks(nchunks, x_r, hi_dst, lo_dst, is_q):
            for c in range(nchunks):
                tagn = "q" if is_q else "k"
                xin = ssb.tile([128, 512], F32, tag="xin", name=f"xin_{tagn}{c}")
                nc.sync.dma_start(
                    xin[:].rearrange("p (t d) -> p t d", d=D),
                    x_r[:, 4 * c:4 * c + 4, :],
                )
                tp = sps.tile([128, 512], F32, tag="tp", bufs=3, name=f"tp_{tagn}{c}")
                for j in range(4):
                    nc.tensor.transpose(
                        tp[:, ts(j, 128)], xin[:, ts(j, 128)], ident[:]
                    )
                xt = ssb.tile([128, 512], F32R, tag="xt", name=f"xt_{tagn}{c}")
                nc.scalar.activation(xt[:], tp[:], AF.Identity)
                hps = sps.tile([128, 512], F32, tag="hps", bufs=2, name=f"hps_{tagn}{c}")
                nc.tensor.matmul(hps[:], wt_sb[:], xt[:], start=True, stop=True)
                hf = ssb.tile([128, 512], F32, tag="hf", name=f"hf_{tagn}{c}")
                if is_q:  # g = -2*(W@xT) - 2b
                    nc.scalar.activation(
                        hf[:], hps[:], AF.Identity, bias=bm2_sb[:, 0:1],
                        scale=-2.0,
                    )
                else:
                    nc.scalar.activation(
                        hf[:], hps[:], AF.Identity, bias=b_sb[:, 0:1]
                    )
                # hi/lo split of h (or g)
                nc.gpsimd.tensor_copy(hi_dst[:, ts(c, 512)], hf[:])
                nc.vector.tensor_tensor(
                    lo_dst[:, ts(c, 512)], hf[:], hi_dst[:, ts(c, 512)],
                    ALU.subtract,
                )
                # squared norms, also hi/lo so the K=128 sum keeps f32 grade
                s2f = ssb.tile([128, 512], F32, tag="s2f", name=f"s2f_{tagn}{c}")
                nc.vector.tensor_mul(s2f[:], hf[:], hf[:])
                sqps = sps.tile([128, 512], F32, tag="sqps", bufs=2, name=f"sqps_{tagn}{c}")
                # plain-f32 matmul (2-pass internally) keeps the norm exact
                nc.tensor.matmul(
                    sqps[0:1, :], ones_col[:], s2f[:], start=True, stop=True
                )
                if is_q:  # raw 4*|h|^2; the 1/4 scale is applied at transpose
                    nc.scalar.activation(
                        sqq_row[0:1, ts(c, 512)], sqps[0:1, :], AF.Identity
                    )
                else:
                    nc.scalar.activation(
                        aug_k[0:1, ts(c, 512)], sqps[0:1, :], AF.Identity
                    )
                    sk = rows_pool.tile([1, 512], F32R, tag="sklo", bufs=2,
                                        name=f"sklo{c}")
                    nc.vector.tensor_tensor(
                        sk[:], sqps[0:1, :], aug_k[0:1, ts(c, 512)],
                        ALU.subtract,
                    )
                    nc.sync.dma_start(aug_k[1:2, ts(c, 512)], sk[:])

        do_chunks(NQ // 512, xq_r, gq_hi, gq_lo, True)
        # transpose sq_q row into column-per-qtile layout via tiny PE transposes
        sqq_ps = sps.tile([128, 512], F32, tag="sqps", bufs=2, name="sqq_ps")
        for qt in range(NQT):
            nc.tensor.transpose(
                sqq_ps[:, qt:qt + 1], sqq_row[0:1, ts(qt, 128)], ident[0:1, 0:1]
            )
        nc.vector.tensor_scalar_mul(sqq_cols[:], sqq_ps[:, 0:NQT], 0.25)
        do_chunks(NK // 512, xk_r, hk_hi, hk_lo, False)

    # ---------------- main loop ----------------
    stage_pool = ctx.enter_context(tc.tile_pool(name="stage", bufs=8))
    d2_ps = ctx.enter_context(tc.tile_pool(name="d2", bufs=4, space="PSUM"))
    NC = NK // 512  # 8 key chunks

    last_act = [None]

    def chained_act(*args, chain=True, **kwargs):
        bi = nc.scalar.activation(*args, **kwargs)
        if chain and last_act[0] is not None:
            # arg order: (waiter, dependency) - this op waits on the previous
            add_dep_helper(bi.ins, last_act[0].ins, sync=False,
                           reason="act-table-order")
        last_act[0] = bi
        return bi

    NH = NK // 1024  # 4 slots of 2 key-chunks each
    spans = [(0, 3), (3, 6), (6, 9), (9, 12), (12, 14), (14, 16)]
    assert spans[-1][1] == NQT
    for g0, g1 in spans:
        group = []
        for qt in range(g0, g1):
            st = stage_pool.tile([128, NK], F32, tag="st", name=f"st{qt}")
            for h in range(NH):
                ps = d2_ps.tile([128, 1024], F32, tag="d2", name=f"d2_{qt}_{h}")
                # chunk-major: finish each 512-column before moving on, so the
                # slot completes after 8 back-to-back matmuls
                for cc in range(2):
                    c = 2 * h + cc
                    nc.tensor.matmul(
                        ps[:, ts(cc, 512)], gq_hi[:, ts(qt, 128)],
                        hk_hi[:, ts(c, 512)], start=True, stop=False,
                    )
                    nc.tensor.matmul(
                        ps[:, ts(cc, 512)], gq_hi[:, ts(qt, 128)],
                        hk_lo[:, ts(c, 512)], start=False, stop=False,
                    )
                    nc.tensor.matmul(
                        ps[:, ts(cc, 512)], gq_lo[:, ts(qt, 128)],
                        hk_hi[:, ts(c, 512)], start=False, stop=False,
                    )
                    nc.tensor.matmul(
                        ps[:, ts(cc, 512)], ones2[:],
                        aug_k[:, ts(c, 512)], start=False, stop=True,
                    )
                nc.vector.tensor_scalar(
                    st[:, ts(h, 1024)], ps[:], sqq_cols[:, qt:qt + 1], 0.0,
                    ALU.add, ALU.max,
                )
            if g0 == 0 and qt < 3:
                # pipeline-fill phase: sqrt per 2048-half starts ~2 slots earlier
                chained_act(st[:, 0:2048], st[:, 0:2048], AF.Sqrt)
                chained_act(st[:, 2048:4096], st[:, 2048:4096], AF.Sqrt)
            else:
                chained_act(st[:], st[:], AF.Sqrt)
            group.append((qt, st))
        for qt, st in group:
            if qt == NQT - 1:
                # final tile: halve exp+DMA so the last DMA overlaps the exp
                chained_act(st[:, 0:2048], st[:, 0:2048], AF.Exp,
                            scale=-TEMPERATURE)
                nc.sync.dma_start(out[ts(qt, 128), 0:2048], st[:, 0:2048])
                chained_act(st[:, 2048:4096], st[:, 2048:4096], AF.Exp,
                            scale=-TEMPERATURE)
                nc.sync.dma_start(out[ts(qt, 128), 2048:4096], st[:, 2048:4096])
            else:
                chained_act(st[:], st[:], AF.Exp, scale=-TEMPERATURE)
                nc.sync.dma_start(out[ts(qt, 128), :], st[:])


def build_nc():
    nc = bacc.Bacc("TRN2", target_bir_lowering=False, debug=False)
    xq = nc.dram_tensor("xq", [NQ, D], F32, kind="ExternalInput").ap()
    xk = nc.dram_tensor("xk", [NK, D], F32, kind="ExternalInput").ap()
    W = nc.dram_tensor("W", [D, D], F32, kind="ExternalInput").ap()
    b = nc.dram_tensor("b", [D, 1], F32, kind="ExternalInput").ap()
    out = nc.dram_tensor("out", [NQ, NK], F32, kind="ExternalOutput").ap()
    with tile.TileContext(nc) as tc:
        with ExitStack() as ctx:
            kernel_body(ctx, tc, out, xq, xk, W, b)
    nc.compile()
    return nc


_NC_CACHE = None


def _get_nc():
    global _NC_CACHE
    if _NC_CACHE is None:
        _NC_CACHE = build_nc()
    return _NC_CACHE


def _run(x, W, b, trace=False, **spmd_kwargs):
    from concourse.bass_utils import run_bass_kernel_spmd

    x = np.asarray(x, dtype=np.float32)
    W = np.asarray(W, dtype=np.float32)
    b = np.asarray(b, dtype=np.float32).reshape(D, 1)
    nc = _get_nc()
    in_maps = []
    for c in range(N_CORES):
        bi, qh = c // 2, c % 2
        in_maps.append({
            "xq": np.ascontiguousarray(x[bi, qh * NQ:(qh + 1) * NQ, :]),
            "xk": np.ascontiguousarray(x[bi]),
            "W": W,
            "b": b,
        })
    res = run_bass_kernel_spmd(
        nc, in_maps, core_ids=list(range(N_CORES)), trace=trace, **spmd_kwargs
    )
    out = np.empty((B, N, N), dtype=np.float32)
    for c in range(N_CORES):
        bi, qh = c // 2, c % 2
        out[bi, qh * NQ:(qh + 1) * NQ, :] = res.results[c]["out"]
    return out, res


def kernel(x, W, b):
    out, _ = _run(x, W, b)
    return out
